# revision 21
# baseline (speedup 1.0000x reference)
"""Pairwise squared Euclidean distance dist[i,j] = ||s_i - t_j||^2 on 8
Trainium2 NeuronCores.

Full inputs s [8192, 512] f32, t [8192, 512] f32 -> dist [8192, 8192] f32.

Strategy: dist = s_sq[:,None] + t_sq[None,:] - 2 s @ t^T.
2D shard over the 8 cores: 4 s-row blocks x 2 t-row blocks; each core
computes a [2048, 4096] output block via a local fp32r GEMM:
  psum = (-2 s_blk) @ t_blk^T             (TensorE, fp32r, k-tiled by 128)
  out  = (psum + s_sq[i]) + t_sq[j]       (one VectorE scalar_tensor_tensor)
Host transposes the blocks (contraction dim must be on partitions) and
precomputes the row norms; t_sq rows are broadcast across partitions
on-device by GpSimd. Inputs stream in n-slices so the PE starts ~2 us in;
outputs buffer in SBUF (ot pool) so the DMA engines stay saturated.
"""
from contextlib import ExitStack

import numpy as np

import concourse.bacc as bacc
import concourse.tile as tile
from concourse import mybir
from concourse.bass_utils import run_bass_kernel_spmd

F32 = mybir.dt.float32
F32R = mybir.dt.float32r

N_S, N_T, D = 8192, 8192, 512      # full problem shape (hardcoded)
SB, TB = 4, 2                      # s-blocks x t-blocks = 8 cores
MS, NS = N_S // SB, N_T // TB      # per-core block: 2048 x 4096
KT = D // 128                      # 4 k-tiles
MT = MS // 128                     # 16 m-tiles
NT = NS // 512                     # 8 n-tiles

_CACHE = {}


def _build():
    nc = bacc.Bacc("TRN2", target_bir_lowering=False, debug=False, num_devices=8)
    sT_ap = nc.dram_tensor("sT", [KT, 128, MS], F32R, kind="ExternalInput").ap()
    tT_ap = nc.dram_tensor("tT", [KT, 128, NS], F32R, kind="ExternalInput").ap()
    ssq_ap = nc.dram_tensor("ssq", [128, MT], F32, kind="ExternalInput").ap()
    tsq_ap = nc.dram_tensor("tsq", [1, NS], F32, kind="ExternalInput").ap()
    out_ap = nc.dram_tensor("out", [MS, NS], F32, kind="ExternalOutput").ap()

    with tile.TileContext(nc) as tc, ExitStack() as ctx:
        w_pool = ctx.enter_context(tc.tile_pool(name="w", bufs=1))
        r_pool = ctx.enter_context(tc.tile_pool(name="r", bufs=NT // 2))
        q_pool = ctx.enter_context(tc.tile_pool(name="q", bufs=NT // 2))
        c_pool = ctx.enter_context(tc.tile_pool(name="c", bufs=1))
        ot_pool = ctx.enter_context(tc.tile_pool(name="ot", bufs=16))
        ps_pool = ctx.enter_context(tc.tile_pool(name="ps", bufs=8, space="PSUM"))

        # sT resident [128, MS] per k-tile, loaded in 512-column chunks so the
        # first matmuls can start after ~3 us instead of waiting for all 4 MB.
        sT_sb = [
            w_pool.tile([128, MS], F32R, tag=f"w{k}", name=f"w{k}")
            for k in range(KT)
        ]
        NP = NT // 2                   # n-pairs; out/in DMAs cover 1024 cols
        r_tiles = [[None] * KT for _ in range(NP)]
        tq_tiles = [None] * NP

        def load_pair(p, interleave_sT_first_col=False):
            psl = slice(p * 1024, (p + 1) * 1024)
            for k in range(KT):
                if interleave_sT_first_col:
                    # (sT[k] m=0 col, r[k]) pairs: the k-th matmul of the
                    # very first group unblocks after 2(k+1) DMAs
                    nc.sync.dma_start(
                        out=sT_sb[k][:, 0:128], in_=sT_ap[k][:, 0:128]
                    )
                r = r_pool.tile([128, 1024], F32R, tag=f"r{k}", name=f"r{k}")
                nc.sync.dma_start(out=r[:], in_=tT_ap[k][:, psl])
                r_tiles[p][k] = r
            tr = q_pool.tile([1, 1024], F32, tag="tr", name="tr")
            nc.sync.dma_start(out=tr[:], in_=tsq_ap[:, psl])
            tq = q_pool.tile([128, 1024], F32, tag="tq", name="tq")
            nc.gpsimd.partition_broadcast(tq[:], tr[:])
            tq_tiles[p] = tq

        with tc.high_priority():
            load_pair(0, interleave_sT_first_col=True)
            for k in range(KT):
                nc.sync.dma_start(out=sT_sb[k][:, 128:512], in_=sT_ap[k][:, 128:512])
            for c in range(1, MT // 4):
                csl = slice(c * 512, (c + 1) * 512)
                for k in range(KT):
                    nc.sync.dma_start(out=sT_sb[k][:, csl], in_=sT_ap[k][:, csl])
            ssq_sb = c_pool.tile([128, MT], F32, tag="ssq")
            nc.sync.dma_start(out=ssq_sb[:], in_=ssq_ap[:])
            for p in range(1, NP):
                load_pair(p)

        for p in range(NP):
            r_sb = r_tiles[p]
            tq = tq_tiles[p]
            for m in range(MT):
                ot = ot_pool.tile([128, 1024], F32, tag="ot", name="ot")
                for h in range(2):
                    hsl = slice(h * 512, (h + 1) * 512)
                    ps = ps_pool.tile([128, 512], F32, tag="ps", name="ps")
                    for k in range(KT):
                        nc.tensor.matmul(
                            ps[:],
                            lhsT=sT_sb[k][:, m * 128:(m + 1) * 128],
                            rhs=r_sb[k][:, hsl],
                            start=(k == 0),
                            stop=(k == KT - 1),
                        )
                    # ot = (psum + ssq[m]) + tsq  -- whole epilogue, one DVE op
                    nc.vector.scalar_tensor_tensor(
                        ot[:, hsl],
                        ps[:],
                        ssq_sb[:, m:m + 1],
                        tq[:, hsl],
                        op0=mybir.AluOpType.add,
                        op1=mybir.AluOpType.add,
                    )
                nc.sync.dma_start(
                    out=out_ap[m * 128:(m + 1) * 128, p * 1024:(p + 1) * 1024],
                    in_=ot[:],
                )
    nc.compile()
    return nc


def _prep_in_maps(s: np.ndarray, t: np.ndarray) -> list[dict[str, np.ndarray]]:
    ssq_full = np.einsum("ij,ij->i", s.astype(np.float64), s.astype(np.float64))
    tsq_full = np.einsum("ij,ij->i", t.astype(np.float64), t.astype(np.float64))
    in_maps = []
    for c in range(8):
        si, tj = c // TB, c % TB
        s_blk = s[si * MS:(si + 1) * MS]
        t_blk = t[tj * NS:(tj + 1) * NS]
        sT = np.ascontiguousarray((-2.0 * s_blk).T.reshape(KT, 128, MS))
        tT = np.ascontiguousarray(t_blk.T.reshape(KT, 128, NS))
        ssq = ssq_full[si * MS:(si + 1) * MS].astype(np.float32)
        tsq = tsq_full[tj * NS:(tj + 1) * NS].astype(np.float32)
        in_maps.append({
            "sT": sT,
            "tT": tT,
            "ssq": np.ascontiguousarray(ssq.reshape(MT, 128).T),
            "tsq": np.ascontiguousarray(tsq.reshape(1, NS)),
        })
    return in_maps


def _run(s: np.ndarray, t: np.ndarray, trace: bool = False, tmpdir=None):
    if "nc" not in _CACHE:
        _CACHE["nc"] = _build()
    nc = _CACHE["nc"]
    in_maps = _prep_in_maps(s, t)
    res = run_bass_kernel_spmd(
        nc, in_maps, core_ids=list(range(8)), trace=trace, tmpdir=tmpdir
    )
    out = np.empty((N_S, N_T), dtype=np.float32)
    for c in range(8):
        si, tj = c // TB, c % TB
        out[si * MS:(si + 1) * MS, tj * NS:(tj + 1) * NS] = res.results[c]["out"]
    return out, res


def kernel(s: np.ndarray, t: np.ndarray) -> np.ndarray:
    s = np.ascontiguousarray(np.asarray(s, dtype=np.float32))
    t = np.ascontiguousarray(np.asarray(t, dtype=np.float32))
    assert s.shape == (N_S, D) and t.shape == (N_T, D)
    out, _ = _run(s, t)
    return out


def bench(s: np.ndarray, t: np.ndarray, iters: int = 8, reps: int = 3):
    """Time the NEFF execution: chain `iters` sequential executions inside one
    jit (outputs feed the next call's output buffers, forcing sequential
    dependency), so per-exec time = slope, free of dispatch latency."""
    import time

    import jax
    import jax.numpy as jnp
    from jax.sharding import Mesh, PartitionSpec
    from jax.experimental.shard_map import shard_map

    from concourse import mybir as _mybir
    from concourse.bass2jax import (
        _bass_exec_p,
        install_neuronx_cc_hook,
        partition_id_tensor,
    )

    install_neuronx_cc_hook()
    if "nc" not in _CACHE:
        _CACHE["nc"] = _build()
    nc = _CACHE["nc"]
    in_maps = _prep_in_maps(s, t)

    partition_name = nc.partition_id_tensor.name if nc.partition_id_tensor else None
    in_names, out_names, out_avals, zero_outs = [], [], [], []
    for alloc in nc.m.functions[0].allocations:
        if not isinstance(alloc, _mybir.MemoryLocationSet):
            continue
        name = alloc.memorylocations[0].name
        if alloc.kind == "ExternalInput":
            if name != partition_name:
                in_names.append(name)
        elif alloc.kind == "ExternalOutput":
            out_names.append(name)
            shape = tuple(alloc.tensor_shape)
            dtype = _mybir.dt.np(alloc.dtype)
            out_avals.append(jax.core.ShapedArray(shape, dtype))
            zero_outs.append(np.zeros(shape, dtype))
    n_params = len(in_names)
    n_outs = len(out_avals)
    all_in_names = list(in_names) + list(out_names)
    if partition_name is not None:
        all_in_names.append(partition_name)

    def body(*args):
        operands = list(args)
        if partition_name is not None:
            operands.append(partition_id_tensor())
        return tuple(
            _bass_exec_p.bind(
                *operands,
                out_avals=tuple(out_avals),
                in_names=tuple(all_in_names),
                out_names=tuple(out_names),
                lowering_input_output_aliases=(),
                sim_require_finite=True,
                sim_require_nnan=True,
                nc=nc,
            )
        )

    devices = jax.devices()[:8]
    mesh = Mesh(np.asarray(devices), ("core",))
    in_specs = (PartitionSpec("core"),) * (n_params + n_outs)
    out_specs = (PartitionSpec("core"),) * n_outs
    donate = tuple(range(n_params, n_params + n_outs))
    fn = jax.jit(
        shard_map(body, mesh=mesh, in_specs=in_specs, out_specs=out_specs,
                  check_rep=False),
        donate_argnums=donate,
        keep_unused=True,
    )

    per_core = [[np.asarray(m[name]) for name in in_names] for m in in_maps]
    concat_in = [
        np.concatenate([per_core[c][i] for c in range(8)], axis=0)
        for i in range(n_params)
    ]
    sharding = jax.sharding.NamedSharding(mesh, PartitionSpec("core"))
    ins_dev = [jax.device_put(a, sharding) for a in concat_in]

    def make_zeros():
        return [
            jax.device_put(
                np.zeros((8 * z.shape[0], *z.shape[1:]), z.dtype), sharding
            )
            for z in zero_outs
        ]

    # compile + warm
    out = fn(*ins_dev, *make_zeros())
    jax.block_until_ready(out)

    # Chain executions: exec i's outputs are exec i+1's donated output-buffer
    # operands, forcing device-side serialization (data dependency). Marginal
    # slope between k_lo and k_hi cancels fixed sync cost; per-call dispatch
    # is tiny (~45 us) and pipelines under the serialized device work.
    k_lo, k_hi = max(2, iters // 4), iters
    totals = {k_lo: [], k_hi: []}
    for _ in range(reps):
        for k in (k_lo, k_hi):
            outs = make_zeros()
            jax.block_until_ready(outs)
            t0 = time.perf_counter()
            for _ in range(k):
                outs = list(fn(*ins_dev, *outs))
            jax.block_until_ready(outs)
            totals[k].append(time.perf_counter() - t0)
    t_lo, t_hi = min(totals[k_lo]), min(totals[k_hi])
    per_exec_ns = (t_hi - t_lo) / (k_hi - k_lo) * 1e9
    return per_exec_ns, {
        f"total_k{k_lo}": t_lo,
        f"total_k{k_hi}": t_hi,
        "amortized_hi": t_hi / k_hi,
    }


# revision 28
# speedup vs baseline: 1.1797x; 1.1797x over previous
"""Pairwise squared Euclidean distance dist[i,j] = ||s_i - t_j||^2 on 8
Trainium2 NeuronCores.

Full inputs s [8192, 512] f32, t [8192, 512] f32 -> dist [8192, 8192] f32.

Strategy: dist = s_sq[:,None] + t_sq[None,:] - 2 s @ t^T.
2D shard over the 8 cores: 4 s-row blocks x 2 t-row blocks; each core
computes a [2048, 4096] output block via a local fp32r GEMM:
  psum = (-2 s_blk) @ t_blk^T             (TensorE, fp32r, k-tiled by 128)
  out  = (psum + s_sq[i]) + t_sq[j]       (one VectorE scalar_tensor_tensor)
Host transposes the blocks (contraction dim must be on partitions) and
precomputes the row norms; t_sq rows are broadcast across partitions
on-device by GpSimd. Inputs stream in n-slices so the PE starts ~2 us in;
outputs buffer in SBUF (ot pool) so the DMA engines stay saturated.
"""
from contextlib import ExitStack

import numpy as np

import concourse.bacc as bacc
import concourse.tile as tile
from concourse import mybir
from concourse.bass_utils import run_bass_kernel_spmd

F32 = mybir.dt.float32
F32R = mybir.dt.float32r

N_S, N_T, D = 8192, 8192, 512      # full problem shape (hardcoded)
SB, TB = 4, 2                      # s-blocks x t-blocks = 8 cores
MS, NS = N_S // SB, N_T // TB      # per-core block: 2048 x 4096
KT = D // 128                      # 4 k-tiles
MT = MS // 128                     # 16 m-tiles
NT = NS // 512                     # 8 n-tiles

_CACHE = {}


def _build(repeat: int = 1):
    """Build the per-core program. repeat>1 re-emits the whole body that many
    times inside one NEFF -- used only for benchmark timing (slope between
    repeat counts isolates one body's pure HW time)."""
    nc = bacc.Bacc("TRN2", target_bir_lowering=False, debug=False, num_devices=8)
    sT_ap = nc.dram_tensor("sT", [KT, 128, MS], F32R, kind="ExternalInput").ap()
    tT_ap = nc.dram_tensor("tT", [KT, 128, NS], F32R, kind="ExternalInput").ap()
    ssq_ap = nc.dram_tensor("ssq", [128, MT], F32, kind="ExternalInput").ap()
    tsq_ap = nc.dram_tensor("tsq", [1, NS], F32, kind="ExternalInput").ap()
    out_ap = nc.dram_tensor("out", [MS, NS], F32, kind="ExternalOutput").ap()

    with tile.TileContext(nc) as tc, ExitStack() as ctx:
        w_pool = ctx.enter_context(tc.tile_pool(name="w", bufs=1))
        r_pool = ctx.enter_context(tc.tile_pool(name="r", bufs=NT // 2))
        q_pool = ctx.enter_context(tc.tile_pool(name="q", bufs=3))
        c_pool = ctx.enter_context(tc.tile_pool(name="c", bufs=1))
        ot_pool = ctx.enter_context(tc.tile_pool(name="ot", bufs=20))
        ps_pool = ctx.enter_context(tc.tile_pool(name="ps", bufs=8, space="PSUM"))

        NP = NT // 2                   # n-pairs; out/in DMAs cover 1024 cols
        for _rep in range(repeat):
            # sT resident [128, MS] per k-tile, loaded in 512-column chunks so
            # the first matmuls start after ~2 us instead of waiting for 4 MB.
            sT_sb = [
                w_pool.tile([128, MS], F32R, tag=f"w{k}", name=f"w{k}")
                for k in range(KT)
            ]
            r_tiles = [[None] * KT for _ in range(NP)]
            tq_tiles = [None] * NP

            def load_pair(p, interleave_sT_first_col=False):
                psl = slice(p * 1024, (p + 1) * 1024)
                for k in range(KT):
                    r = r_pool.tile([128, 1024], F32R, tag=f"r{k}", name=f"r{k}")
                    if interleave_sT_first_col:
                        # (sT[k] m=0 col, r[k]) pairs: the k-th matmul of the
                        # very first group unblocks after 2(k+1) DMAs
                        nc.sync.dma_start(
                            out=sT_sb[k][:, 0:128], in_=sT_ap[k][:, 0:128]
                        )
                    nc.sync.dma_start(out=r[:], in_=tT_ap[k][:, psl])
                    r_tiles[p][k] = r
                tr = q_pool.tile([1, 1024], F32, tag="tr", name="tr")
                nc.sync.dma_start(out=tr[:], in_=tsq_ap[:, psl])
                tq = q_pool.tile([128, 1024], F32, tag="tq", name="tq")
                nc.gpsimd.partition_broadcast(tq[:], tr[:])
                tq_tiles[p] = tq

            with tc.high_priority(offset=None if _rep == 0 else 0):
                load_pair(0, interleave_sT_first_col=True)
                ssq_sb = c_pool.tile([128, MT], F32, tag="ssq", name="ssq")
                nc.sync.dma_start(out=ssq_sb[:], in_=ssq_ap[:])
                for k in range(KT):
                    nc.sync.dma_start(
                        out=sT_sb[k][:, 128:512], in_=sT_ap[k][:, 128:512]
                    )
                for c in range(1, MT // 4):
                    csl = slice(c * 512, (c + 1) * 512)
                    for k in range(KT):
                        nc.sync.dma_start(out=sT_sb[k][:, csl], in_=sT_ap[k][:, csl])
                for p in range(1, NP):
                    load_pair(p)

            for p in range(NP):
                r_sb = r_tiles[p]
                tq = tq_tiles[p]
                for m in range(MT):
                    ot = ot_pool.tile([128, 1024], F32, tag="ot", name="ot")
                    for h in range(2):
                        hsl = slice(h * 512, (h + 1) * 512)
                        ps = ps_pool.tile([128, 512], F32, tag="ps", name="ps")
                        for k in range(KT):
                            nc.tensor.matmul(
                                ps[:],
                                lhsT=sT_sb[k][:, m * 128:(m + 1) * 128],
                                rhs=r_sb[k][:, hsl],
                                start=(k == 0),
                                stop=(k == KT - 1),
                            )
                        # ot = (psum + ssq[m]) + tsq -- whole epilogue, 1 DVE op
                        nc.vector.scalar_tensor_tensor(
                            ot[:, hsl],
                            ps[:],
                            ssq_sb[:, m:m + 1],
                            tq[:, hsl],
                            op0=mybir.AluOpType.add,
                            op1=mybir.AluOpType.add,
                        )
                    nc.sync.dma_start(
                        out=out_ap[m * 128:(m + 1) * 128, p * 1024:(p + 1) * 1024],
                        in_=ot[:],
                    )
    nc.compile()
    return nc


def _prep_in_maps(s: np.ndarray, t: np.ndarray) -> list[dict[str, np.ndarray]]:
    ssq_full = np.einsum("ij,ij->i", s.astype(np.float64), s.astype(np.float64))
    tsq_full = np.einsum("ij,ij->i", t.astype(np.float64), t.astype(np.float64))
    in_maps = []
    for c in range(8):
        si, tj = c // TB, c % TB
        s_blk = s[si * MS:(si + 1) * MS]
        t_blk = t[tj * NS:(tj + 1) * NS]
        sT = np.ascontiguousarray((-2.0 * s_blk).T.reshape(KT, 128, MS))
        tT = np.ascontiguousarray(t_blk.T.reshape(KT, 128, NS))
        ssq = ssq_full[si * MS:(si + 1) * MS].astype(np.float32)
        tsq = tsq_full[tj * NS:(tj + 1) * NS].astype(np.float32)
        in_maps.append({
            "sT": sT,
            "tT": tT,
            "ssq": np.ascontiguousarray(ssq.reshape(MT, 128).T),
            "tsq": np.ascontiguousarray(tsq.reshape(1, NS)),
        })
    return in_maps


def _run(s: np.ndarray, t: np.ndarray, trace: bool = False, tmpdir=None):
    if "nc" not in _CACHE:
        _CACHE["nc"] = _build()
    nc = _CACHE["nc"]
    in_maps = _prep_in_maps(s, t)
    res = run_bass_kernel_spmd(
        nc, in_maps, core_ids=list(range(8)), trace=trace, tmpdir=tmpdir
    )
    out = np.empty((N_S, N_T), dtype=np.float32)
    for c in range(8):
        si, tj = c // TB, c % TB
        out[si * MS:(si + 1) * MS, tj * NS:(tj + 1) * NS] = res.results[c]["out"]
    return out, res


def kernel(s: np.ndarray, t: np.ndarray) -> np.ndarray:
    s = np.ascontiguousarray(np.asarray(s, dtype=np.float32))
    t = np.ascontiguousarray(np.asarray(t, dtype=np.float32))
    assert s.shape == (N_S, D) and t.shape == (N_T, D)
    out, _ = _run(s, t)
    return out


def bench(s: np.ndarray, t: np.ndarray, iters: int = 8, reps: int = 3):
    """Time the NEFF execution: chain `iters` sequential executions inside one
    jit (outputs feed the next call's output buffers, forcing sequential
    dependency), so per-exec time = slope, free of dispatch latency."""
    import time

    import jax
    import jax.numpy as jnp
    from jax.sharding import Mesh, PartitionSpec
    from jax.experimental.shard_map import shard_map

    from concourse import mybir as _mybir
    from concourse.bass2jax import (
        _bass_exec_p,
        install_neuronx_cc_hook,
        partition_id_tensor,
    )

    install_neuronx_cc_hook()
    if "nc" not in _CACHE:
        _CACHE["nc"] = _build()
    nc = _CACHE["nc"]
    in_maps = _prep_in_maps(s, t)

    partition_name = nc.partition_id_tensor.name if nc.partition_id_tensor else None
    in_names, out_names, out_avals, zero_outs = [], [], [], []
    for alloc in nc.m.functions[0].allocations:
        if not isinstance(alloc, _mybir.MemoryLocationSet):
            continue
        name = alloc.memorylocations[0].name
        if alloc.kind == "ExternalInput":
            if name != partition_name:
                in_names.append(name)
        elif alloc.kind == "ExternalOutput":
            out_names.append(name)
            shape = tuple(alloc.tensor_shape)
            dtype = _mybir.dt.np(alloc.dtype)
            out_avals.append(jax.core.ShapedArray(shape, dtype))
            zero_outs.append(np.zeros(shape, dtype))
    n_params = len(in_names)
    n_outs = len(out_avals)
    all_in_names = list(in_names) + list(out_names)
    if partition_name is not None:
        all_in_names.append(partition_name)

    def body(*args):
        operands = list(args)
        if partition_name is not None:
            operands.append(partition_id_tensor())
        return tuple(
            _bass_exec_p.bind(
                *operands,
                out_avals=tuple(out_avals),
                in_names=tuple(all_in_names),
                out_names=tuple(out_names),
                lowering_input_output_aliases=(),
                sim_require_finite=True,
                sim_require_nnan=True,
                nc=nc,
            )
        )

    devices = jax.devices()[:8]
    mesh = Mesh(np.asarray(devices), ("core",))
    in_specs = (PartitionSpec("core"),) * (n_params + n_outs)
    out_specs = (PartitionSpec("core"),) * n_outs
    donate = tuple(range(n_params, n_params + n_outs))
    fn = jax.jit(
        shard_map(body, mesh=mesh, in_specs=in_specs, out_specs=out_specs,
                  check_rep=False),
        donate_argnums=donate,
        keep_unused=True,
    )

    per_core = [[np.asarray(m[name]) for name in in_names] for m in in_maps]
    concat_in = [
        np.concatenate([per_core[c][i] for c in range(8)], axis=0)
        for i in range(n_params)
    ]
    sharding = jax.sharding.NamedSharding(mesh, PartitionSpec("core"))
    ins_dev = [jax.device_put(a, sharding) for a in concat_in]

    def make_zeros():
        return [
            jax.device_put(
                np.zeros((8 * z.shape[0], *z.shape[1:]), z.dtype), sharding
            )
            for z in zero_outs
        ]

    # compile + warm
    out = fn(*ins_dev, *make_zeros())
    jax.block_until_ready(out)

    # Chain executions: exec i's outputs are exec i+1's donated output-buffer
    # operands, forcing device-side serialization (data dependency). Marginal
    # slope between k_lo and k_hi cancels fixed sync cost; per-call dispatch
    # is tiny (~45 us) and pipelines under the serialized device work.
    k_lo, k_hi = max(2, iters // 4), iters
    totals = {k_lo: [], k_hi: []}
    for _ in range(reps):
        for k in (k_lo, k_hi):
            outs = make_zeros()
            jax.block_until_ready(outs)
            t0 = time.perf_counter()
            for _ in range(k):
                outs = list(fn(*ins_dev, *outs))
            jax.block_until_ready(outs)
            totals[k].append(time.perf_counter() - t0)
    t_lo, t_hi = min(totals[k_lo]), min(totals[k_hi])
    per_exec_ns = (t_hi - t_lo) / (k_hi - k_lo) * 1e9
    return per_exec_ns, {
        f"total_k{k_lo}": t_lo,
        f"total_k{k_hi}": t_hi,
        "amortized_hi": t_hi / k_hi,
    }
